# revision 21
# baseline (speedup 1.0000x reference)
"""Cross-attention kernel for Trainium2, 8 NeuronCores, data-parallel over batch.

Problem (per batch element b, one per core):
    q  = x_b @ Wq.T + bq                      [T=1024, C=1024]
    kv = enc_b @ Wkv.T + bkv                  [I=576, 2C]
    per head h (H=16, D=64):
        att = softmax((q_h @ k_h.T) / sqrt(D))
        y_h = att @ v_h
    out = y @ Wo.T + bo                       [T, C]

Design notes (v2):
  - One batch element per core (B=8 == n_cores), no collectives.
  - All transposes are done on HOST: x^T / enc^T / W^T arrive pre-laid-out
    as [128, 8, N] bf16 so the contraction dim (c) is on SBUF partitions.
    No PE transposes, no PSUM round-trips for layout.
  - All matmul operands are bf16 (cast on host); PSUM accumulation stays
    f32.  End-to-end rel err ~7e-3 (vs 2e-2 tolerance).
  - Each weight is DMA'd exactly once with 16KB-contiguous partition lines.
  - Attention: S^T = K_h @ Q_h^T per head in [i, t] orientation; exp without
    max-subtraction into bf16; one ACT instruction per [i-chunk, 1024] (both
    t-halves).  The softmax denominator Z falls out of the AV matmul via a
    ones column in V (lhsT M=65).  1/Z via reciprocal_approx_fast straight
    off the PSUM Z row into a resident [16, T] table; normalization is a
    rank-2 PE broadcast (2 heads at once) + one DVE multiply, deferred a
    few heads to stay off the critical path (keeps HAM un-throttled).
  - Biases: bq/bk are per-partition adds; bv/bo are rank-1 (K=1) matmul
    accumulates of ones^T (x) bias_row.
"""

import numpy as np
import ml_dtypes

T = 1024
C = 1024
I = 576
H = 16
D = 64
NCC = C // 128          # 8 contraction chunks
NIC = (I + 127) // 128  # 5 i chunks (128,128,128,128,64)
I_CH = [128, 128, 128, 128, 64]
VW = 68                 # per-head column block in V tile: 64 v cols + ones col + pad
SCALE = 1.0 / np.sqrt(D)

_CACHE = {}


def _build_nc():
    import concourse.bass as bass
    import concourse.bacc as bacc
    import concourse.mybir as mybir
    import concourse.tile as tile
    from concourse.dve_ops import RECIP_APPROX_FAST_CONSTS, RECIPROCAL_APPROX_FAST
    from contextlib import ExitStack

    f32 = mybir.dt.float32
    f32r = mybir.dt.float32r
    bf16 = mybir.dt.bfloat16

    nc = bacc.Bacc()

    # host-pre-transposed inputs: [p, cc, n] with c = cc*128 + p on partitions
    xt_d = nc.dram_tensor("xt", [128, NCC, T], bf16, kind="ExternalInput")
    ect_d = nc.dram_tensor("ect", [128, NCC, I], bf16, kind="ExternalInput")
    wq_d = nc.dram_tensor("wq", [128, NCC, C], bf16, kind="ExternalInput")
    wk_d = nc.dram_tensor("wk", [128, NCC, C], bf16, kind="ExternalInput")
    wv_d = nc.dram_tensor("wv", [128, NCC, C], bf16, kind="ExternalInput")
    wo_d = nc.dram_tensor("wo", [128, NCC, C], bf16, kind="ExternalInput")
    bq_d = nc.dram_tensor("bqp", [128, NCC], f32, kind="ExternalInput")
    bk_d = nc.dram_tensor("bkp", [128, NCC], f32, kind="ExternalInput")
    bv_d = nc.dram_tensor("bv", [C], bf16, kind="ExternalInput")
    bo_d = nc.dram_tensor("bo", [C], bf16, kind="ExternalInput")
    out_d = nc.dram_tensor("out", [T, C], f32, kind="ExternalOutput")

    # consts
    vones_d = nc.inline_tensor(np.ones((128, 16), dtype=ml_dtypes.bfloat16), name="vones_d")
    onesr_d = nc.inline_tensor(np.ones((1, 128), dtype=ml_dtypes.bfloat16), name="onesr_d")
    # sel4c[z, j, m]: picks Z rows {0,32} (j=0) or {64,96} (j=1) into row-halves
    sel4_np = np.zeros((128, 2, 128), dtype=np.float32)
    sel4_np[0, 0, 0:64] = 1.0
    sel4_np[32, 0, 64:128] = 1.0
    sel4_np[64, 1, 0:64] = 1.0
    sel4_np[96, 1, 64:128] = 1.0
    sel4_d = nc.inline_tensor(sel4_np, name="sel4_d")

    with ExitStack() as ctx:
        tc = ctx.enter_context(tile.TileContext(nc))

        resid = ctx.enter_context(tc.tile_pool(name="resid", bufs=1))
        misc = ctx.enter_context(tc.tile_pool(name="misc", bufs=1))
        exps = ctx.enter_context(tc.tile_pool(name="exps", bufs=13))

        # consts + biases (DMAs issued after the first x/w chunks below)
        ones_r = misc.tile([1, 128], bf16)
        sel4 = misc.tile([128, 2, 128], f32r)
        bq_t = misc.tile([128, NCC], f32)
        bk_t = misc.tile([128, NCC], f32)
        bv_row = misc.tile([1, C], bf16)
        bo_row = misc.tile([1, C], bf16)

        # resident tensors
        QT = [resid.tile([128, T], bf16, tag=f"QT{i}", name=f"QT{i}") for i in range(NCC)]
        KT = [resid.tile([128, I], bf16, tag=f"KT{i}", name=f"KT{i}") for i in range(NCC)]
        V3 = [resid.tile([128, H, VW], bf16, tag=f"V{i}", name=f"V{i}") for i in range(NIC)]
        YTu = [resid.tile([128, T], bf16, tag=f"YTu{i}", name=f"YTu{i}") for i in range(NCC)]
        YT = YTu  # normalized in place
        Zc = [resid.tile([128, 512], f32, tag=f"Zc{i}", name=f"Zc{i}") for i in range(8)]
        nrs = resid.tile([128, 512], f32, tag="nrs", name="nrs")
        Zi = [resid.tile([128, 512], f32r, tag=f"Zi{i}", name=f"Zi{i}") for i in range(8)]

        # staged inputs that live until the out-projection
        stage = ctx.enter_context(tc.tile_pool(name="stage", bufs=1))
        ect = stage.tile([128, NCC, I], bf16, tag="ect", name="ect")
        wkt = stage.tile([128, NCC, C], bf16, tag="wkt", name="wkt")
        wvt = stage.tile([128, NCC, C], bf16, tag="wvt", name="wvt")
        wot = stage.tile([128, NCC, C], bf16, tag="wot", name="wot")

        with tc.tile_pool(name="ph1", bufs=1) as ph1:
            # wq/xt chunked so the first Q-proj matmuls start as soon as
            # chunk 0 lands; remaining stage DMAs follow in first-use order
            xt = ph1.tile([128, NCC, T], bf16, tag="xt", name="xt")
            wqt = ph1.tile([128, NCC, C], bf16, tag="wqt", name="wqt")
            for cc in range(NCC):
                nc.sync.dma_start(out=wqt[:, cc, :], in_=wq_d[:, cc, :])
                nc.sync.dma_start(out=xt[:, cc, :], in_=xt_d[:, cc, :])
                if cc == 1:
                    nc.sync.dma_start(out=bq_t, in_=bq_d[:, :])
                    nc.sync.dma_start(out=bk_t, in_=bk_d[:, :])
            nc.sync.dma_start(out=ect, in_=ect_d[:, :, :])
            nc.sync.dma_start(out=wkt, in_=wk_d[:, :, :])
            nc.sync.dma_start(out=wvt, in_=wv_d[:, :, :])
            nc.sync.dma_start(out=ones_r, in_=onesr_d[:, :])
            nc.sync.dma_start(out=sel4, in_=sel4_d[:, :, :].bitcast(f32r))
            nc.sync.dma_start(out=bv_row, in_=bv_d[:].unsqueeze(0))
            nc.sync.dma_start(out=bo_row, in_=bo_d[:].unsqueeze(0))
            nc.sync.dma_start(out=wot, in_=wo_d[:, :, :])
            for ii in range(NIC):
                nc.gpsimd.memset(V3[ii][:, :, 64:65], 1.0)
            for k in range(8):
                nc.gpsimd.memset(Zc[k][:, :], 1.0)

            # ---- Q^T projection: QT[oc][o_p, t] ----
            with tc.tile_pool(name="qp", bufs=6, space="PSUM") as qp:
              for oc in range(NCC):
                for th in range(2):
                    pq = qp.tile([128, 512], f32, tag="qp", name="pq")
                    for cc in range(NCC):
                        nc.tensor.matmul(
                            pq,
                            wqt[:, cc, oc * 128 : (oc + 1) * 128],
                            xt[:, cc, th * 512 : (th + 1) * 512],
                            start=(cc == 0),
                            stop=(cc == NCC - 1),
                        )
                    nc.vector.tensor_scalar_add(
                        QT[oc][:, th * 512 : (th + 1) * 512],
                        pq,
                        bq_t[:, oc : oc + 1],
                    )

        # ---- interleaved K/V projection + attention ----
        pp = tc.alloc_tile_pool(name="pp", bufs=1, space="PSUM")
        psp = tc.alloc_tile_pool(name="psp", bufs=2, space="PSUM")
        pyp = tc.alloc_tile_pool(name="pyp", bufs=2, space="PSUM")
        pbp = tc.alloc_tile_pool(name="pbp", bufs=1, space="PSUM")
        # K(oc) unlocks S for heads 2oc/2oc+1; exp (scalar engine) starts
        # grinding while the PE is still doing projections, so the scalar
        # engine's ~88us hides inside the PE's dense work and HAM stays warm.
        def emit_k(oc):
            for ih in range(2):
                pk = pp.tile([128, 512], f32, tag="pp", name="pk")
                for cc in range(NCC):
                    nc.tensor.matmul(
                        pk[:, :288],
                        wkt[:, cc, oc * 128 : (oc + 1) * 128],
                        ect[:, cc, ih * 288 : (ih + 1) * 288],
                        start=(cc == 0),
                        stop=(cc == NCC - 1),
                    )
                nc.vector.tensor_scalar_add(
                    KT[oc][:, ih * 288 : (ih + 1) * 288],
                    pk[:, :288],
                    bk_t[:, oc : oc + 1],
                )

        def emit_v(och):
            for ii in range(NIC):
                pi = I_CH[ii]
                pv = pp.tile([128, 512], f32, tag="pp", name="pv")
                for cc in range(NCC):
                    nc.tensor.matmul(
                        pv[:pi],
                        ect[:, cc, ii * 128 : ii * 128 + pi],
                        wvt[:, cc, och * 512 : (och + 1) * 512],
                        start=(cc == 0),
                        stop=False,
                    )
                nc.tensor.matmul(
                    pv[:pi],
                    ones_r[0:1, :pi],
                    bv_row[0:1, och * 512 : (och + 1) * 512],
                    start=False,
                    stop=True,
                )
                dst = V3[ii][:pi, och * 8 : och * 8 + 8, 0:64]
                nc.vector.tensor_copy(
                    dst, pv[:pi].rearrange("p (h d) -> p h d", d=64)
                )

        es_tiles = {}

        def emit_s(h):
            oc = h // 2
            hb = (h % 2) * 64
            for ii in range(NIC):
                pi = I_CH[ii]
                ps = psp.tile([128, 1024], f32, tag="ps", name="ps")
                for tch in range(2):
                    nc.tensor.matmul(
                        ps[:pi, tch * 512 : (tch + 1) * 512],
                        KT[oc][hb : hb + 64, ii * 128 : ii * 128 + pi],
                        QT[oc][hb : hb + 64, tch * 512 : (tch + 1) * 512],
                        start=True,
                        stop=True,
                    )
                e = exps.tile([128, 1024], bf16, tag="es", name="es")
                nc.scalar.activation(
                    e[:pi],
                    ps[:pi],
                    mybir.ActivationFunctionType.Exp,
                    scale=float(SCALE),
                )
                es_tiles[(h, ii)] = e

        def emit_av(h, tch):
            oc = h // 2
            hb = (h % 2) * 64
            tsl = slice(tch * 512, (tch + 1) * 512)
            py = pyp.tile([128, 512], f32, tag="py", name="py")
            for ii in range(NIC):
                pi = I_CH[ii]
                e = es_tiles[(h, ii)] if tch == 0 else es_tiles.pop((h, ii))
                nc.tensor.matmul(
                    py[:65],
                    V3[ii][:pi, h, 0:65],
                    e[:pi, tsl],
                    start=(ii == 0),
                    stop=(ii == NIC - 1),
                )
            nc.vector.tensor_copy(YTu[oc][hb : hb + 64, tsl], py[0:64])
            # Z row (partition 64) -> 32-aligned row of collection tile
            k = h // 4 + 4 * tch
            row = 64 * ((h // 2) % 2) + 32 * (h % 2)
            nc.vector.tensor_copy(Zc[k][row : row + 1], py[64:65])

        def emit_recip(q):
            for k in (q, q + 4):
                with nc.allow_low_precision(reason="1/Z in f32r is fine"):
                    nc.vector.reciprocal(Zi[k], Zc[k])

        def emit_norm(p):
            # normalize pair p (heads 2p, 2p+1); rows at 64*(p%2) + {0,32}
            j = p % 2
            for tch in range(2):
                tsl = slice(tch * 512, (tch + 1) * 512)
                k = p // 2 + 4 * tch
                pb = pbp.tile([128, 512], f32, tag="pb", name="pb")
                nc.tensor.matmul(
                    pb,
                    sel4[:, j, :],
                    Zi[k][:, :],
                    start=True,
                    stop=True,
                )
                nc.vector.tensor_mul(YT[p][:, tsl], YTu[p][:, tsl], pb)

        def post_av(h):
            if h % 4 == 3:
                emit_recip(h // 4)
            if h in (6, 7, 10, 11, 14, 15):
                emit_norm({6: 0, 7: 1, 10: 2, 11: 3, 14: 4, 15: 5}[h])

        emit_k(0)
        emit_k(1)
        emit_s(0)
        emit_s(1)
        emit_v(0)
        def emit_v1(och, iis):
            for ii in iis:
                pi = I_CH[ii]
                pv = pp.tile([128, 512], f32, tag="pp", name="pv")
                for cc in range(NCC):
                    nc.tensor.matmul(
                        pv[:pi],
                        ect[:, cc, ii * 128 : ii * 128 + pi],
                        wvt[:, cc, och * 512 : (och + 1) * 512],
                        start=(cc == 0),
                        stop=False,
                    )
                nc.tensor.matmul(
                    pv[:pi],
                    ones_r[0:1, :pi],
                    bv_row[0:1, och * 512 : (och + 1) * 512],
                    start=False,
                    stop=True,
                )
                dst = V3[ii][:pi, och * 8 : och * 8 + 8, 0:64]
                nc.vector.tensor_copy(
                    dst, pv[:pi].rearrange("p (h d) -> p h d", d=64)
                )

        for h in range(H):
            if h + 2 <= 7:
                emit_k(h + 2)
            if h == 5:
                emit_v1(1, (0, 1, 2))
            if h == 6:
                emit_v1(1, (3, 4))
            emit_av(h, 0)
            emit_av(h, 1)
            if h + 2 < H:
                emit_s(h + 2)
            post_av(h)
        emit_norm(6)
        emit_norm(7)
        pbp.release()
        pyp.release()
        psp.release()
        pp.release()

        # ---- output projection ----
        with tc.tile_pool(name="osb", bufs=2) as osb, \
             tc.tile_pool(name="op", bufs=6, space="PSUM") as op:
            for tt in range(8):
                ot = osb.tile([128, C], f32, tag="osb", name="ot")
                for och in range(2):
                    po = op.tile([128, 512], f32, tag="op", name="po")
                    for cc in range(NCC):
                        nc.tensor.matmul(
                            po,
                            YT[cc][:, tt * 128 : (tt + 1) * 128],
                            wot[:, cc, och * 512 : (och + 1) * 512],
                            start=(cc == 0),
                            stop=False,
                        )
                    nc.tensor.matmul(
                        po,
                        ones_r[0:1, 0:128],
                        bo_row[0:1, och * 512 : (och + 1) * 512],
                        start=False,
                        stop=True,
                    )
                    nc.vector.tensor_copy(ot[:, och * 512 : (och + 1) * 512], po)
                    nc.sync.dma_start(
                        out=out_d[
                            tt * 128 : (tt + 1) * 128, och * 512 : (och + 1) * 512
                        ],
                        in_=ot[:, och * 512 : (och + 1) * 512],
                    )

    nc.compile()
    return nc


def _get_nc():
    if "nc" not in _CACHE:
        _CACHE["nc"] = _build_nc()
    return _CACHE["nc"]


def _to_chunked_bf16(a):
    # [R, N] f32 (R = 1024 rows of the contraction dim) -> [128, R//128, N] bf16
    r, n = a.shape
    return np.ascontiguousarray(
        a.reshape(r // 128, 128, n).transpose(1, 0, 2)
    ).astype(ml_dtypes.bfloat16)


def _prep_in_maps(x, encoder_output, Wq, bq, Wkv, bkv, Wo, bo):
    f = np.float32
    bf = ml_dtypes.bfloat16
    x = np.asarray(x, f)
    enc = np.asarray(encoder_output, f)
    Wq = np.asarray(Wq, f)
    wkv = np.asarray(Wkv, f)
    Wo = np.asarray(Wo, f)
    bq = np.asarray(bq, f)
    bkv = np.asarray(bkv, f)
    bo = np.asarray(bo, f)
    shared = {
        "wq": _to_chunked_bf16(np.ascontiguousarray(Wq.T)),
        "wk": _to_chunked_bf16(np.ascontiguousarray(wkv[:C].T)),
        "wv": _to_chunked_bf16(np.ascontiguousarray(wkv[C:].T)),
        "wo": _to_chunked_bf16(np.ascontiguousarray(Wo.T)),
        "bqp": np.ascontiguousarray(bq.reshape(NCC, 128).T),
        "bkp": np.ascontiguousarray(bkv[:C].reshape(NCC, 128).T),
        "bv": np.ascontiguousarray(bkv[C:]).astype(bf),
        "bo": np.ascontiguousarray(bo).astype(bf),
    }
    return [
        dict(
            shared,
            xt=_to_chunked_bf16(np.ascontiguousarray(x[b].T)),
            ect=_to_chunked_bf16(np.ascontiguousarray(enc[b].T)),
        )
        for b in range(x.shape[0])
    ]


def kernel(x, encoder_output, Wq, bq, Wkv, bkv, Wo, bo):
    from concourse.bass_utils import run_bass_kernel_spmd

    nc = _get_nc()
    in_maps = _prep_in_maps(x, encoder_output, Wq, bq, Wkv, bkv, Wo, bo)
    res = run_bass_kernel_spmd(nc, in_maps, list(range(len(in_maps)))).results
    return np.stack([res[b]["out"] for b in range(len(res))]).astype(np.float32)


# revision 24
# speedup vs baseline: 1.1946x; 1.1946x over previous
"""Cross-attention kernel for Trainium2, 8 NeuronCores, data-parallel over batch.

Problem (per batch element b, one per core):
    q  = x_b @ Wq.T + bq                      [T=1024, C=1024]
    kv = enc_b @ Wkv.T + bkv                  [I=576, 2C]
    per head h (H=16, D=64):
        att = softmax((q_h @ k_h.T) / sqrt(D))
        y_h = att @ v_h
    out = y @ Wo.T + bo                       [T, C]

Design notes (v2):
  - One batch element per core (B=8 == n_cores), no collectives.
  - All transposes are done on HOST: x^T / enc^T / W^T arrive pre-laid-out
    as [128, 8, N] bf16 so the contraction dim (c) is on SBUF partitions.
    No PE transposes, no PSUM round-trips for layout.
  - All matmul operands are bf16 (cast on host); PSUM accumulation stays
    f32.  End-to-end rel err ~7e-3 (vs 2e-2 tolerance).
  - Each weight is DMA'd exactly once with 16KB-contiguous partition lines.
  - Attention: S^T = K_h @ Q_h^T per head in [i, t] orientation; exp without
    max-subtraction into bf16; one ACT instruction per [i-chunk, 1024] (both
    t-halves).  The softmax denominator Z falls out of the AV matmul via a
    ones column in V (lhsT M=65).  1/Z via reciprocal_approx_fast straight
    off the PSUM Z row into a resident [16, T] table; normalization is a
    rank-2 PE broadcast (2 heads at once) + one DVE multiply, deferred a
    few heads to stay off the critical path (keeps HAM un-throttled).
  - Biases: bq/bk are per-partition adds; bv/bo are rank-1 (K=1) matmul
    accumulates of ones^T (x) bias_row.
"""

import numpy as np
import ml_dtypes

T = 1024
C = 1024
I = 576
H = 16
D = 64
NCC = C // 128          # 8 contraction chunks
NIC = (I + 127) // 128  # 5 i chunks (128,128,128,128,64)
I_CH = [128, 128, 128, 128, 64]
VW = 68                 # per-head column block in V tile: 64 v cols + ones col + pad
SCALE = 1.0 / np.sqrt(D)

_CACHE = {}


def _build_nc():
    import concourse.bass as bass
    import concourse.bacc as bacc
    import concourse.mybir as mybir
    import concourse.tile as tile
    from concourse.dve_ops import RECIP_APPROX_FAST_CONSTS, RECIPROCAL_APPROX_FAST
    from contextlib import ExitStack

    f32 = mybir.dt.float32
    f32r = mybir.dt.float32r
    bf16 = mybir.dt.bfloat16

    nc = bacc.Bacc()

    # host-pre-transposed inputs: [p, cc, n] with c = cc*128 + p on partitions
    xt_d = nc.dram_tensor("xt", [128, NCC, T], bf16, kind="ExternalInput")
    ect_d = nc.dram_tensor("ect", [128, NCC, I], bf16, kind="ExternalInput")
    wq_d = nc.dram_tensor("wq", [128, NCC, C], bf16, kind="ExternalInput")
    wk_d = nc.dram_tensor("wk", [128, NCC, C], bf16, kind="ExternalInput")
    wv_d = nc.dram_tensor("wv", [128, NCC, C], bf16, kind="ExternalInput")
    wo_d = nc.dram_tensor("wo", [128, NCC, C], bf16, kind="ExternalInput")
    bq_d = nc.dram_tensor("bqp", [128, NCC], f32, kind="ExternalInput")
    bk_d = nc.dram_tensor("bkp", [128, NCC], f32, kind="ExternalInput")
    bv_d = nc.dram_tensor("bv", [C], bf16, kind="ExternalInput")
    bo_d = nc.dram_tensor("bo", [C], bf16, kind="ExternalInput")
    out_d = nc.dram_tensor("out", [T, C], f32, kind="ExternalOutput")

    # consts
    vones_d = nc.inline_tensor(np.ones((128, 16), dtype=ml_dtypes.bfloat16), name="vones_d")
    onesr_d = nc.inline_tensor(np.ones((1, 128), dtype=ml_dtypes.bfloat16), name="onesr_d")
    # sel4c[z, j, m]: picks Z rows {0,32} (j=0) or {64,96} (j=1) into row-halves
    sel4_np = np.zeros((128, 2, 128), dtype=np.float32)
    sel4_np[0, 0, 0:64] = 1.0
    sel4_np[32, 0, 64:128] = 1.0
    sel4_np[64, 1, 0:64] = 1.0
    sel4_np[96, 1, 64:128] = 1.0
    sel4_d = nc.inline_tensor(sel4_np, name="sel4_d")

    with ExitStack() as ctx:
        tc = ctx.enter_context(tile.TileContext(nc))

        resid = ctx.enter_context(tc.tile_pool(name="resid", bufs=1))
        misc = ctx.enter_context(tc.tile_pool(name="misc", bufs=1))
        exps = ctx.enter_context(tc.tile_pool(name="exps", bufs=14))

        # consts + biases (DMAs issued after the first x/w chunks below)
        ones_r = misc.tile([1, 128], bf16)
        sel4 = misc.tile([128, 2, 128], f32r)
        bq_t = misc.tile([128, NCC], f32)
        bk_t = misc.tile([128, NCC], f32)
        bv_row = misc.tile([1, C], bf16)
        bo_row = misc.tile([1, C], bf16)

        # resident tensors
        QT = [resid.tile([128, T], bf16, tag=f"QT{i}", name=f"QT{i}") for i in range(NCC)]
        KT = [resid.tile([128, I], bf16, tag=f"KT{i}", name=f"KT{i}") for i in range(NCC)]
        V3 = [resid.tile([128, H, VW], bf16, tag=f"V{i}", name=f"V{i}") for i in range(NIC)]
        YTu = [resid.tile([128, T], bf16, tag=f"YTu{i}", name=f"YTu{i}") for i in range(NCC)]
        YT = YTu  # normalized in place
        Zc = [resid.tile([128, 512], f32, tag=f"Zc{i}", name=f"Zc{i}") for i in range(8)]
        Zi = [resid.tile([128, 512], f32r, tag=f"Zi{i}", name=f"Zi{i}") for i in range(8)]

        # staged inputs that live until the out-projection
        stage = ctx.enter_context(tc.tile_pool(name="stage", bufs=1))
        ect = stage.tile([128, NCC, I], bf16, tag="ect", name="ect")
        wkt = stage.tile([128, NCC, C], bf16, tag="wkt", name="wkt")
        wvt = stage.tile([128, NCC, C], bf16, tag="wvt", name="wvt")
        wot = stage.tile([128, NCC, C], bf16, tag="wot", name="wot")

        with tc.tile_pool(name="ph1", bufs=1) as ph1:
            # wq/xt chunked so the first Q-proj matmuls start as soon as
            # chunk 0 lands; remaining stage DMAs follow in first-use order
            xt = ph1.tile([128, NCC, T], bf16, tag="xt", name="xt")
            wqt = ph1.tile([128, NCC, C], bf16, tag="wqt", name="wqt")
            for cc in range(NCC):
                nc.sync.dma_start(out=wqt[:, cc, :], in_=wq_d[:, cc, :])
                nc.sync.dma_start(out=xt[:, cc, :], in_=xt_d[:, cc, :])
                if cc == 1:
                    nc.sync.dma_start(out=bq_t, in_=bq_d[:, :])
                    nc.sync.dma_start(out=bk_t, in_=bk_d[:, :])
            nc.sync.dma_start(out=ect, in_=ect_d[:, :, :])
            nc.sync.dma_start(out=wkt, in_=wk_d[:, :, :])
            nc.sync.dma_start(out=wvt, in_=wv_d[:, :, :])
            nc.sync.dma_start(out=ones_r, in_=onesr_d[:, :])
            nc.sync.dma_start(out=sel4, in_=sel4_d[:, :, :].bitcast(f32r))
            nc.sync.dma_start(out=bv_row, in_=bv_d[:].unsqueeze(0))
            nc.sync.dma_start(out=bo_row, in_=bo_d[:].unsqueeze(0))
            nc.sync.dma_start(out=wot, in_=wo_d[:, :, :])
            for ii in range(NIC):
                nc.gpsimd.memset(V3[ii][:, :, 64:65], 1.0)
            for k in range(8):
                nc.gpsimd.memset(Zc[k][:, :], 1.0)

            # ---- Q^T projection: QT[oc][o_p, t] ----
            with tc.tile_pool(name="qp", bufs=6, space="PSUM") as qp:
              for oc in range(NCC):
                for th in range(2):
                    pq = qp.tile([128, 512], f32, tag="qp", name="pq")
                    for cc in range(NCC):
                        nc.tensor.matmul(
                            pq,
                            wqt[:, cc, oc * 128 : (oc + 1) * 128],
                            xt[:, cc, th * 512 : (th + 1) * 512],
                            start=(cc == 0),
                            stop=(cc == NCC - 1),
                        )
                    nc.vector.tensor_scalar_add(
                        QT[oc][:, th * 512 : (th + 1) * 512],
                        pq,
                        bq_t[:, oc : oc + 1],
                    )

        # ---- interleaved K/V projection + attention ----
        pp = tc.alloc_tile_pool(name="pp", bufs=1, space="PSUM")
        psp = tc.alloc_tile_pool(name="psp", bufs=2, space="PSUM")
        pyp = tc.alloc_tile_pool(name="pyp", bufs=2, space="PSUM")
        pbp = tc.alloc_tile_pool(name="pbp", bufs=1, space="PSUM")
        # K(oc) unlocks S for heads 2oc/2oc+1; exp (scalar engine) starts
        # grinding while the PE is still doing projections, so the scalar
        # engine's ~88us hides inside the PE's dense work and HAM stays warm.
        def emit_k(oc):
            for ih in range(2):
                pk = pp.tile([128, 512], f32, tag="pp", name="pk")
                for cc in range(NCC):
                    nc.tensor.matmul(
                        pk[:, :288],
                        wkt[:, cc, oc * 128 : (oc + 1) * 128],
                        ect[:, cc, ih * 288 : (ih + 1) * 288],
                        start=(cc == 0),
                        stop=(cc == NCC - 1),
                    )
                nc.vector.tensor_scalar_add(
                    KT[oc][:, ih * 288 : (ih + 1) * 288],
                    pk[:, :288],
                    bk_t[:, oc : oc + 1],
                )

        def emit_v(och):
            for ii in range(NIC):
                pi = I_CH[ii]
                pv = pp.tile([128, 512], f32, tag="pp", name="pv")
                for cc in range(NCC):
                    nc.tensor.matmul(
                        pv[:pi],
                        ect[:, cc, ii * 128 : ii * 128 + pi],
                        wvt[:, cc, och * 512 : (och + 1) * 512],
                        start=(cc == 0),
                        stop=False,
                    )
                nc.tensor.matmul(
                    pv[:pi],
                    ones_r[0:1, :pi],
                    bv_row[0:1, och * 512 : (och + 1) * 512],
                    start=False,
                    stop=True,
                )
                dst = V3[ii][:pi, och * 8 : och * 8 + 8, 0:64]
                nc.vector.tensor_copy(
                    dst, pv[:pi].rearrange("p (h d) -> p h d", d=64)
                )

        es_tiles = {}

        def emit_s(h):
            oc = h // 2
            hb = (h % 2) * 64
            for ii in range(NIC):
                pi = I_CH[ii]
                ps = psp.tile([128, 1024], f32, tag="ps", name="ps")
                for tch in range(2):
                    nc.tensor.matmul(
                        ps[:pi, tch * 512 : (tch + 1) * 512],
                        KT[oc][hb : hb + 64, ii * 128 : ii * 128 + pi],
                        QT[oc][hb : hb + 64, tch * 512 : (tch + 1) * 512],
                        start=True,
                        stop=True,
                    )
                e = exps.tile([128, 1024], bf16, tag="es", name="es")
                nc.scalar.activation(
                    e[:pi],
                    ps[:pi],
                    mybir.ActivationFunctionType.Exp,
                    scale=float(SCALE),
                )
                es_tiles[(h, ii)] = e

        def emit_av(h, tch):
            oc = h // 2
            hb = (h % 2) * 64
            tsl = slice(tch * 512, (tch + 1) * 512)
            py = pyp.tile([128, 512], f32, tag="py", name="py")
            for ii in range(NIC):
                pi = I_CH[ii]
                e = es_tiles[(h, ii)] if tch == 0 else es_tiles.pop((h, ii))
                nc.tensor.matmul(
                    py[:65],
                    V3[ii][:pi, h, 0:65],
                    e[:pi, tsl],
                    start=(ii == 0),
                    stop=(ii == NIC - 1),
                )
            nc.vector.tensor_copy(YTu[oc][hb : hb + 64, tsl], py[0:64])
            # Z row (partition 64) -> 32-aligned row of collection tile
            k = h // 4 + 4 * tch
            row = 64 * ((h // 2) % 2) + 32 * (h % 2)
            nc.vector.tensor_copy(Zc[k][row : row + 1], py[64:65])

        def emit_recip(q):
            for k in (q, q + 4):
                with nc.allow_low_precision(reason="1/Z in f32r is fine"):
                    nc.vector.reciprocal(Zi[k], Zc[k])

        def emit_norm(p):
            # normalize pair p (heads 2p, 2p+1); rows at 64*(p%2) + {0,32}
            j = p % 2
            for tch in range(2):
                tsl = slice(tch * 512, (tch + 1) * 512)
                k = p // 2 + 4 * tch
                pb = pbp.tile([128, 512], f32, tag="pb", name="pb")
                nc.tensor.matmul(
                    pb,
                    sel4[:, j, :],
                    Zi[k][:, :],
                    start=True,
                    stop=True,
                )
                nc.vector.tensor_mul(YT[p][:, tsl], YTu[p][:, tsl], pb)

        def post_av(h):
            if h % 4 == 3:
                emit_recip(h // 4)
            if h in (5, 6, 9, 10, 13, 14):
                emit_norm({5: 0, 6: 1, 9: 2, 10: 3, 13: 4, 14: 5}[h])

        emit_k(0)
        emit_k(1)
        emit_s(0)
        emit_s(1)
        emit_v(0)
        def emit_v1(och, iis):
            for ii in iis:
                pi = I_CH[ii]
                pv = pp.tile([128, 512], f32, tag="pp", name="pv")
                for cc in range(NCC):
                    nc.tensor.matmul(
                        pv[:pi],
                        ect[:, cc, ii * 128 : ii * 128 + pi],
                        wvt[:, cc, och * 512 : (och + 1) * 512],
                        start=(cc == 0),
                        stop=False,
                    )
                nc.tensor.matmul(
                    pv[:pi],
                    ones_r[0:1, :pi],
                    bv_row[0:1, och * 512 : (och + 1) * 512],
                    start=False,
                    stop=True,
                )
                dst = V3[ii][:pi, och * 8 : och * 8 + 8, 0:64]
                nc.vector.tensor_copy(
                    dst, pv[:pi].rearrange("p (h d) -> p h d", d=64)
                )

        for h in range(H):
            if h + 2 <= 7:
                emit_k(h + 2)
            if h == 3:
                emit_v1(1, (0, 1, 2, 3, 4))
            emit_av(h, 0)
            emit_av(h, 1)
            if h + 2 < H:
                emit_s(h + 2)
            post_av(h)
        emit_norm(6)
        emit_norm(7)
        pbp.release()
        pyp.release()
        psp.release()
        pp.release()

        # ---- output projection ----
        with tc.tile_pool(name="osb", bufs=2) as osb, \
             tc.tile_pool(name="op", bufs=6, space="PSUM") as op:
            for tt in range(8):
                ot = osb.tile([128, C], f32, tag="osb", name="ot")
                for och in range(2):
                    po = op.tile([128, 512], f32, tag="op", name="po")
                    for cc in range(NCC):
                        nc.tensor.matmul(
                            po,
                            YT[cc][:, tt * 128 : (tt + 1) * 128],
                            wot[:, cc, och * 512 : (och + 1) * 512],
                            start=(cc == 0),
                            stop=False,
                        )
                    nc.tensor.matmul(
                        po,
                        ones_r[0:1, 0:128],
                        bo_row[0:1, och * 512 : (och + 1) * 512],
                        start=False,
                        stop=True,
                    )
                    nc.vector.tensor_copy(ot[:, och * 512 : (och + 1) * 512], po)
                    nc.sync.dma_start(
                        out=out_d[
                            tt * 128 : (tt + 1) * 128, och * 512 : (och + 1) * 512
                        ],
                        in_=ot[:, och * 512 : (och + 1) * 512],
                    )

    nc.compile()
    return nc


def _get_nc():
    if "nc" not in _CACHE:
        _CACHE["nc"] = _build_nc()
    return _CACHE["nc"]


def _to_chunked_bf16(a):
    # [R, N] f32 (R = 1024 rows of the contraction dim) -> [128, R//128, N] bf16
    r, n = a.shape
    return np.ascontiguousarray(
        a.reshape(r // 128, 128, n).transpose(1, 0, 2)
    ).astype(ml_dtypes.bfloat16)


def _prep_in_maps(x, encoder_output, Wq, bq, Wkv, bkv, Wo, bo):
    f = np.float32
    bf = ml_dtypes.bfloat16
    x = np.asarray(x, f)
    enc = np.asarray(encoder_output, f)
    Wq = np.asarray(Wq, f)
    wkv = np.asarray(Wkv, f)
    Wo = np.asarray(Wo, f)
    bq = np.asarray(bq, f)
    bkv = np.asarray(bkv, f)
    bo = np.asarray(bo, f)
    shared = {
        "wq": _to_chunked_bf16(np.ascontiguousarray(Wq.T)),
        "wk": _to_chunked_bf16(np.ascontiguousarray(wkv[:C].T)),
        "wv": _to_chunked_bf16(np.ascontiguousarray(wkv[C:].T)),
        "wo": _to_chunked_bf16(np.ascontiguousarray(Wo.T)),
        "bqp": np.ascontiguousarray(bq.reshape(NCC, 128).T),
        "bkp": np.ascontiguousarray(bkv[:C].reshape(NCC, 128).T),
        "bv": np.ascontiguousarray(bkv[C:]).astype(bf),
        "bo": np.ascontiguousarray(bo).astype(bf),
    }
    return [
        dict(
            shared,
            xt=_to_chunked_bf16(np.ascontiguousarray(x[b].T)),
            ect=_to_chunked_bf16(np.ascontiguousarray(enc[b].T)),
        )
        for b in range(x.shape[0])
    ]


def kernel(x, encoder_output, Wq, bq, Wkv, bkv, Wo, bo):
    from concourse.bass_utils import run_bass_kernel_spmd

    nc = _get_nc()
    in_maps = _prep_in_maps(x, encoder_output, Wq, bq, Wkv, bkv, Wo, bo)
    res = run_bass_kernel_spmd(nc, in_maps, list(range(len(in_maps)))).results
    return np.stack([res[b]["out"] for b in range(len(res))]).astype(np.float32)
